# revision 21
# baseline (speedup 1.0000x reference)
"""Trainium2 Bass kernel for KGETCDA GNN message-passing layer.

Computes, for fixed-structure inputs:
    side    = segment_sum(a_vals[:,None] * ego[a_cols], a_rows, N)
    sum_emb = LeakyReLU((ego + side) @ W1.T + b1)
    bi_emb  = LeakyReLU((ego * side) @ W2.T + b2)
    out     = sum_emb + bi_emb

Strategy (8 NeuronCores, SPMD, full inputs in / full output out):
  - Shard destination rows contiguously: core c owns rows
    [c*N/8, (c+1)*N/8).  Edges partitioned by destination.
  - Per core, edges are sorted by destination and grouped into 128-dest
    "chunks" (49 per core); each 128-edge tile gathers its source rows
    via gpsimd.dma_gather (per-edge descriptor DMA from a fp16 [N,128]
    padded copy of ego) and scatter-reduces into a 128-column slice of a
    [128, 512] PSUM window with one matmul per tile against a [128,128]
    valued one-hot.
  - One-hots for a whole (window, stream) run of tiles are built with
    TWO broadcast-AP tensor_tensor ops (is_equal then val multiply) --
    DVE per-instruction fixed cost (~0.4us) made per-tile tensor_scalar
    builds the old bottleneck.
  - The matmul stationary is the full [128,128] fp16 gather tile
    (zero-padded cols 96:128) so Fast Weight Load kicks in.
  - dma_gather indices are int16, so edges are split into stream A
    (src < 32768) and stream B (src >= 32768, rebased); tiles are laid
    out (window, stream)-major so gather calls batch into long
    single-stream runs.
  - Per-(chunk, stream) tile counts are padded to the max over the 8
    cores so the single SPMD instruction stream is valid for every core.
  - Dense tail feature-major in fp16 (FWL): sumXt = egoT+sideT,
    biXt = egoT*sideT per window, then per-512-node group matmuls against
    bias-augmented W1T/W2T, LeakyReLU + branch add on DVE, one DMA out.
"""

import inspect
import textwrap

import numpy as np
import ml_dtypes

import concourse.bacc as bacc
import concourse.bass as bass
import concourse.mybir as mybir
import concourse.tile as tile
from concourse import bass_utils, library_config

_DMA_GATHER_ANY = {}


def _dma_gather_any(eng, *args, **kwargs):
    """dma_gather with the elem_size_bytes%256 assert relaxed to %64.

    The 256B restriction only applies to the transpose/xbar path; the
    non-transpose SWDGE descriptors carry arbitrary byte lengths.  This
    lets us gather 192B (96 fp16) rows from a 256B-strided table, saving
    25% of gather DMA traffic.
    """
    cls = type(eng)
    fn = _DMA_GATHER_ANY.get(cls)
    if fn is None:
        src = textwrap.dedent(inspect.getsource(cls.dma_gather))
        src = src.replace("elem_size_bytes % 256 == 0",
                          "elem_size_bytes % 64 == 0")
        ns = dict(vars(bass))
        exec(src, ns)
        fn = ns["dma_gather"]
        _DMA_GATHER_ANY[cls] = fn
    return fn(eng, *args, **kwargs)

# ---------------------------------------------------------------- constants
N_NODES = 50000
N_EDGES = 800000
D = 96
DPAD = 128          # fp16 gather element (256B, dma_gather alignment)
NCORES = 8
PER = N_NODES // NCORES          # 6250 dests per core
CHUNK = 128                      # dests per one-hot / matmul slice
NCHUNK = (PER + CHUNK - 1) // CHUNK   # 49 chunks (last short: 106)
WIN = 512                        # dests per PSUM window (8 chunks)
NWIN = (PER + WIN - 1) // WIN    # 13 windows
CPW = WIN // CHUNK               # chunks per window
SPLIT = 32768                    # int16 index limit for dma_gather
GT = 128                         # edges per tile
GE = 96                          # gathered elements per edge (192B of the 256B row)
CT = 8                           # tiles per dma_gather call (ring limit ~1024 idxs)
NQ = 4                           # SWDGE queues
PERPAD = 6272                    # 49 * 128, padded node count per core
NDCH = PERPAD // 128             # dense-tail chunks of 128 nodes
NEG_SLOPE = 0.01
DGRP = 4                         # chunks per dense-tail group

FP16 = mybir.dt.float16
F32 = mybir.dt.float32
I16 = mybir.dt.int16


def _win_chunks(w):
    return range(CPW * w, min(CPW * w + CPW, NCHUNK))


# ---------------------------------------------------------------- host prep
def _preprocess(a_rows, a_cols, a_vals):
    """Per-core edge layout with core-uniform tile counts.

    Edges are bucketed by (core, chunk-of-128-dests, int16 stream) and
    padded so every core shares the per-(chunk, stream) tile count
    T[k, s] (max over cores, >=1).  Tile order is (window, stream,
    chunk)-major so gather calls get long single-stream runs.
    """
    a_rows = np.asarray(a_rows).astype(np.int64)
    a_cols = np.asarray(a_cols).astype(np.int64)
    a_vals = np.asarray(a_vals).astype(np.float32)

    order = np.argsort(a_rows, kind="stable")
    r_s, c_s, v_s = a_rows[order], a_cols[order], a_vals[order]
    core_s = r_s // PER
    dloc_s = r_s % PER
    chunk_s = dloc_s // CHUNK
    col_s = dloc_s % CHUNK
    stream_s = (c_s >= SPLIT).astype(np.int64)

    counts = np.zeros((NCORES, NCHUNK, 2), dtype=np.int64)
    buckets = [[[None, None] for _ in range(NCHUNK)] for _ in range(NCORES)]
    for c in range(NCORES):
        m = core_s == c
        ch, st = chunk_s[m], stream_s[m]
        src, val, col = c_s[m], v_s[m], col_s[m]
        for k in range(NCHUNK):
            mk = ch == k
            for s in range(2):
                mm = mk & (st == s)
                idx = src[mm] - (SPLIT if s else 0)
                buckets[c][k][s] = (idx, val[mm], col[mm])
                counts[c, k, s] = mm.sum()

    T = np.zeros((NCHUNK, 2), dtype=np.int64)
    for k in range(NCHUNK):
        for s in range(2):
            T[k, s] = max(1, int(np.ceil(counts[:, k, s].max() / GT)))

    per_core = []
    for c in range(NCORES):
        idx_parts, val_parts, col_parts = [], [], []
        for w in range(NWIN):
            for s in range(2):
                for k in _win_chunks(w):
                    idx, val, col = buckets[c][k][s]
                    n_pad = int(T[k, s]) * GT
                    pad = n_pad - len(idx)
                    idx_parts.append(np.concatenate([idx, np.zeros(pad, np.int64)]))
                    val_parts.append(np.concatenate([val, np.zeros(pad, np.float32)]))
                    col_parts.append(np.concatenate([col, np.zeros(pad, np.int64)]))
        per_core.append(dict(
            idx=np.concatenate(idx_parts),
            val=np.concatenate(val_parts).astype(np.float32),
            col=np.concatenate(col_parts),
        ))
    return T, per_core


def _build_call_plan(T):
    """Tile order is (window, stream, chunk)-major.  Returns:
    calls: list of (stream, tile_start, n_tiles) single-stream runs <= CT
    tmap:  (chunk, stream, j) -> global tile index
    runs:  (window, stream) -> (tile_start, n_tiles) for one-hot batching
    n_tiles total.
    """
    calls = []
    tmap = {}
    runs = {}
    t = 0
    for w in range(NWIN):
        for s in range(2):
            r0 = t
            for k in _win_chunks(w):
                for j in range(int(T[k, s])):
                    tmap[(k, s, j)] = t
                    t += 1
            runs[(w, s)] = (r0, t - r0)
            done = r0
            while done < t:
                kk = min(CT, t - done)
                calls.append((s, done, kk, w))
                done += kk
    return calls, tmap, runs, t


def _wrap_idx16(idx_all, calls):
    """Per-call 16-partition-wrapped int16 index tiles, concatenated.
    Call k with n_tiles tiles occupies columns [8*tile_start, 8*(start+n))
    of a [128, 8*TT] int16 array (8 cols per tile: 128/16)."""
    TT = len(idx_all) // GT
    out = np.zeros((128, 8 * TT), dtype=np.int16)
    for s, t0, nt, w in calls:
        chunk = idx_all[t0 * GT:(t0 + nt) * GT].astype(np.int16)
        wrapped = chunk.reshape(-1, 16).T          # [16, nt*8]
        out[:, t0 * 8:(t0 + nt) * 8] = np.tile(wrapped, (8, 1))
    return out


# ---------------------------------------------------------------- builder
_CACHE = {}
_LAST_RESULT = None


def _build_program(T, calls, tmap, runs, TT):
    nc = bacc.Bacc("TRN2", target_bir_lowering=False, debug=False,
                   num_devices=NCORES, num_swdge_queues=NQ)

    ego_pad = nc.dram_tensor("ego_pad", [N_NODES, DPAD], FP16, kind="ExternalInput")
    idx16 = nc.dram_tensor("idx16", [128, 8 * TT], I16, kind="ExternalInput")
    vals = nc.dram_tensor("vals", [128, TT], FP16, kind="ExternalInput")
    cols = nc.dram_tensor("cols", [128, TT], FP16, kind="ExternalInput")
    iota = nc.dram_tensor("iota", [128, CHUNK], FP16, kind="ExternalInput")
    egot = nc.dram_tensor("egot", [D + 1, PERPAD], FP16, kind="ExternalInput")
    w1t = nc.dram_tensor("w1t", [D + 1, D], FP16, kind="ExternalInput")
    w2t = nc.dram_tensor("w2t", [D + 1, D], FP16, kind="ExternalInput")
    out = nc.dram_tensor("out", [PERPAD, D], F32, kind="ExternalOutput")

    tile2call = {}
    for ci, (s, t0, nt, w) in enumerate(calls):
        for j in range(nt):
            tile2call[t0 + j] = (ci, j)

    max_run = max(n for (_, n) in runs.values())

    with tile.TileContext(nc) as tc:
        with tc.tile_pool(name="const", bufs=1) as constp, \
             tc.tile_pool(name="gath", bufs=20) as gathp, \
             tc.tile_pool(name="oh", bufs=4) as ohp, \
             tc.tile_pool(name="pw", bufs=3, space="PSUM") as pwp, \
             tc.tile_pool(name="pd", bufs=2, space="PSUM") as pdp, \
             tc.tile_pool(name="act", bufs=2) as actp, \
             tc.tile_pool(name="big", bufs=1) as bigp:

            # ---- idx in four staged loads on the sync HWDGE queue
            # (nothing else on it), so gather call 0 starts after ~10us
            # and the rest streams behind; all other constants go via
            # the scalar engine's HWDGE queue.
            bounds = []
            for b in [runs[(0, 1)][0], runs[(1, 0)][0], runs[(2, 0)][0]]:
                if b not in bounds:
                    bounds.append(b)
            bounds.append(TT)
            idx_tiles = []
            prev = 0
            for b in bounds:
                tl = constp.tile([128, 8 * (b - prev)], I16)
                nc.sync.dma_start(tl[:], idx16[:, 8 * prev:8 * b])
                idx_tiles.append((prev, b, tl))
                prev = b

            def idx_ap(t0, nt):
                for lo, hi, tl in idx_tiles:
                    if lo <= t0 < hi:
                        assert t0 + nt <= hi
                        return tl[:, (t0 - lo) * 8:(t0 - lo + nt) * 8]
                raise AssertionError(t0)

            iota_sb = constp.tile([128, CHUNK], FP16)
            nc.scalar.dma_start(iota_sb[:], iota[:])
            w1t_sb = constp.tile([D + 1, D], FP16)
            nc.scalar.dma_start(w1t_sb[:], w1t[:])
            w2t_sb = constp.tile([D + 1, D], FP16)
            nc.scalar.dma_start(w2t_sb[:], w2t[:])
            val_sb = constp.tile([128, TT], FP16)
            nc.scalar.dma_start(val_sb[:], vals[:])
            col_sb = constp.tile([128, TT], FP16)
            nc.scalar.dma_start(col_sb[:], cols[:])
            # egot streamed per window so window 0's epilogue isn't
            # gated on the full 1.2MB load
            egot_sb = bigp.tile([D + 1, PERPAD], FP16)
            for w in range(NWIN):
                c0 = w * WIN
                ce = min(PERPAD, c0 + WIN)
                nc.scalar.dma_start(egot_sb[:, c0:ce], egot[:, c0:ce])

            sumxt = bigp.tile([D + 1, PERPAD], FP16)
            bixt = bigp.tile([D + 1, PERPAD], FP16)

            # ones rows for the bias augmentation; zero the padded tail
            # columns up front so the dense tail can run per window
            nc.vector.memset(sumxt[D:D + 1, :], 1.0)
            nc.vector.memset(bixt[D:D + 1, :], 1.0)
            if PERPAD > PER:
                nc.vector.memset(sumxt[:D, PER:], 0.0)
                nc.vector.memset(bixt[:D, PER:], 0.0)

            nc.gpsimd.load_library(library_config.mlp)

            # ---- gather calls (issued in order; Tile double-buffers)
            # 256B elems: smaller (192B) elems measured ~40% lower DMA
            # throughput per queue, a net loss.
            gath_tiles = [None] * len(calls)
            for ci, (s, t0, nt, w) in enumerate(calls):
                g = gathp.tile([128, CT, DPAD], FP16, tag="gath")
                src_ap = ego_pad[:SPLIT, :] if s == 0 else ego_pad[SPLIT:, :]
                nc.gpsimd.dma_gather(
                    g[:, :nt, :], src_ap, idx_ap(t0, nt),
                    nt * GT, nt * GT, DPAD, queue_num=ci % NQ,
                )
                gath_tiles[ci] = g

            # ---- per-window: batched one-hot build + per-chunk accumulation
            for w in range(NWIN):
                nd = min(WIN, PER - w * WIN)
                pw = pwp.tile([128, WIN], F32, tag="pw")
                # one-hot stacks, one per stream run
                oh_run = {}
                for s in range(2):
                    r0, rn = runs[(w, s)]
                    oh = ohp.tile([128, max_run, CHUNK], FP16, tag="oh")
                    nc.vector.tensor_tensor(
                        oh[:, :rn, :],
                        iota_sb[:, None, :].broadcast_to([128, rn, CHUNK]),
                        col_sb[:, r0:r0 + rn, None].broadcast_to([128, rn, CHUNK]),
                        mybir.AluOpType.is_equal,
                    )
                    nc.vector.tensor_tensor(
                        oh[:, :rn, :], oh[:, :rn, :],
                        val_sb[:, r0:r0 + rn, None].broadcast_to([128, rn, CHUNK]),
                        mybir.AluOpType.mult,
                    )
                    oh_run[s] = (oh, r0)
                for k in _win_chunks(w):
                    c0 = (k - CPW * w) * CHUNK
                    n_a, n_b = int(T[k, 0]), int(T[k, 1])
                    for jj in range(n_a + n_b):
                        s, j = (0, jj) if jj < n_a else (1, jj - n_a)
                        t = tmap[(k, s, j)]
                        ci, slot = tile2call[t]
                        g = gath_tiles[ci]
                        oh, r0 = oh_run[s]
                        nc.tensor.matmul(
                            pw[:, c0:c0 + CHUNK], g[:, slot, :],
                            oh[:, t - r0, :],
                            start=(jj == 0), stop=(jj == n_a + n_b - 1),
                        )
                # sideT window -> sumXt / biXt (feature-major)
                c0 = w * WIN
                nc.vector.tensor_tensor(
                    sumxt[:D, c0:c0 + nd], egot_sb[:D, c0:c0 + nd], pw[:D, :nd],
                    mybir.AluOpType.add,
                )
                nc.vector.tensor_tensor(
                    bixt[:D, c0:c0 + nd], egot_sb[:D, c0:c0 + nd], pw[:D, :nd],
                    mybir.AluOpType.mult,
                )

                # ---- dense tail for this window's nodes (DGRP=4 dense
                # chunks of 128 = exactly one 512-node window)
                g0 = w * DGRP
                ng = min(DGRP, NDCH - g0)
                p1 = pdp.tile([128, DGRP, D], F32, tag="pd1")
                p2 = pdp.tile([128, DGRP, D], F32, tag="pd2")
                for i in range(ng):
                    c0 = (g0 + i) * 128
                    nc.tensor.matmul(p1[:, i, :], sumxt[:, c0:c0 + 128],
                                     w1t_sb[:], start=True, stop=True)
                    nc.tensor.matmul(p2[:, i, :], bixt[:, c0:c0 + 128],
                                     w2t_sb[:], start=True, stop=True)
                s1 = actp.tile([128, DGRP, D], F32, tag="s1")
                nc.vector.tensor_scalar_mul(s1[:, :ng, :], p1[:, :ng, :], NEG_SLOPE)
                a1 = actp.tile([128, DGRP, D], F32, tag="a1")
                nc.vector.tensor_tensor(a1[:, :ng, :], s1[:, :ng, :], p1[:, :ng, :],
                                        mybir.AluOpType.max)
                s2 = actp.tile([128, DGRP, D], F32, tag="s2")
                nc.vector.tensor_scalar_mul(s2[:, :ng, :], p2[:, :ng, :], NEG_SLOPE)
                a2 = actp.tile([128, DGRP, D], F32, tag="a2")
                nc.vector.tensor_tensor(a2[:, :ng, :], s2[:, :ng, :], p2[:, :ng, :],
                                        mybir.AluOpType.max)
                out_g = actp.tile([128, DGRP, D], F32, tag="outg")
                nc.vector.tensor_tensor(out_g[:, :ng, :], a1[:, :ng, :],
                                        a2[:, :ng, :], mybir.AluOpType.add)
                nc.sync.dma_start(
                    out.rearrange("(k p) f -> p k f", p=128)[:, g0:g0 + ng, :],
                    out_g[:, :ng, :])

    nc.compile()
    return nc


# ---------------------------------------------------------------- entry
def kernel(ego, a_vals, W1, b1, W2, b2, a_rows, a_cols):
    ego = np.asarray(ego, dtype=np.float32)
    a_vals = np.asarray(a_vals, dtype=np.float32)
    W1 = np.asarray(W1, dtype=np.float32)
    b1 = np.asarray(b1, dtype=np.float32)
    W2 = np.asarray(W2, dtype=np.float32)
    b2 = np.asarray(b2, dtype=np.float32)
    a_rows_i = np.asarray(a_rows)
    a_cols_i = np.asarray(a_cols)

    T, per_core = _preprocess(a_rows_i, a_cols_i, a_vals)
    calls, tmap, runs, TT = _build_call_plan(T)

    key = (tuple(T.ravel().tolist()),)
    if key not in _CACHE:
        _CACHE[key] = _build_program(T, calls, tmap, runs, TT)
    nc = _CACHE[key]

    # shared inputs
    ego_pad = np.zeros((N_NODES, DPAD), dtype=np.float16)
    ego_pad[:, :D] = ego.astype(np.float16)
    iota_np = np.tile(np.arange(CHUNK, dtype=np.float32).astype(np.float16),
                      (128, 1))
    w1t_np = np.vstack([W1.T, b1[None, :]]).astype(np.float16)
    w2t_np = np.vstack([W2.T, b2[None, :]]).astype(np.float16)

    in_maps = []
    for c in range(NCORES):
        pc = per_core[c]
        idx16_np = _wrap_idx16(pc["idx"], calls)
        val_np = pc["val"].reshape(TT, GT).T.astype(np.float16)
        col_np = np.ascontiguousarray(
            pc["col"].astype(np.float16).reshape(TT, GT).T)
        egot_np = np.zeros((D + 1, PERPAD), dtype=np.float16)
        egot_np[:D, :PER] = ego[c * PER:(c + 1) * PER].T.astype(np.float16)
        egot_np[D, :] = 1.0
        in_maps.append({
            "ego_pad": ego_pad, "idx16": idx16_np,
            "vals": val_np, "cols": col_np, "iota": iota_np,
            "egot": egot_np, "w1t": w1t_np, "w2t": w2t_np,
        })

    res = bass_utils.run_bass_kernel_spmd(
        nc, in_maps, core_ids=list(range(NCORES)))
    global _LAST_RESULT
    _LAST_RESULT = res

    out = np.empty((N_NODES, D), dtype=np.float32)
    for c in range(NCORES):
        out[c * PER:(c + 1) * PER] = res.results[c]["out"][:PER]
    return out


# revision 23
# speedup vs baseline: 1.0463x; 1.0463x over previous
"""Trainium2 Bass kernel for KGETCDA GNN message-passing layer.

Computes, for fixed-structure inputs:
    side    = segment_sum(a_vals[:,None] * ego[a_cols], a_rows, N)
    sum_emb = LeakyReLU((ego + side) @ W1.T + b1)
    bi_emb  = LeakyReLU((ego * side) @ W2.T + b2)
    out     = sum_emb + bi_emb

Strategy (8 NeuronCores, SPMD, full inputs in / full output out):
  - Shard destination rows contiguously: core c owns rows
    [c*N/8, (c+1)*N/8).  Edges partitioned by destination.
  - Per core, edges are sorted by destination and grouped into 128-dest
    "chunks" (49 per core); each 128-edge tile gathers its source rows
    via gpsimd.dma_gather (per-edge descriptor DMA from a fp16 [N,128]
    padded copy of ego) and scatter-reduces into a 128-column slice of a
    [128, 512] PSUM window with one matmul per tile against a [128,128]
    valued one-hot.
  - One-hots for a whole (window, stream) run of tiles are built with
    TWO broadcast-AP tensor_tensor ops (is_equal then val multiply) --
    DVE per-instruction fixed cost (~0.4us) made per-tile tensor_scalar
    builds the old bottleneck.
  - The matmul stationary is the full [128,128] fp16 gather tile
    (zero-padded cols 96:128) so Fast Weight Load kicks in.
  - dma_gather indices are int16, so edges are split into stream A
    (src < 32768) and stream B (src >= 32768, rebased); tiles are laid
    out (window, stream)-major so gather calls batch into long
    single-stream runs.
  - Per-(chunk, stream) tile counts are padded to the max over the 8
    cores so the single SPMD instruction stream is valid for every core.
  - Dense tail feature-major in fp16 (FWL): sumXt = egoT+sideT,
    biXt = egoT*sideT per window, then per-512-node group matmuls against
    bias-augmented W1T/W2T, LeakyReLU + branch add on DVE, one DMA out.
"""

import inspect
import textwrap

import numpy as np
import ml_dtypes

import concourse.bacc as bacc
import concourse.bass as bass
import concourse.mybir as mybir
import concourse.tile as tile
from concourse import bass_utils, library_config

_DMA_GATHER_ANY = {}


def _dma_gather_any(eng, *args, **kwargs):
    """dma_gather with the elem_size_bytes%256 assert relaxed to %64.

    The 256B restriction only applies to the transpose/xbar path; the
    non-transpose SWDGE descriptors carry arbitrary byte lengths.  This
    lets us gather 192B (96 fp16) rows from a 256B-strided table, saving
    25% of gather DMA traffic.
    """
    cls = type(eng)
    fn = _DMA_GATHER_ANY.get(cls)
    if fn is None:
        src = textwrap.dedent(inspect.getsource(cls.dma_gather))
        src = src.replace("elem_size_bytes % 256 == 0",
                          "elem_size_bytes % 64 == 0")
        ns = dict(vars(bass))
        exec(src, ns)
        fn = ns["dma_gather"]
        _DMA_GATHER_ANY[cls] = fn
    return fn(eng, *args, **kwargs)

# ---------------------------------------------------------------- constants
N_NODES = 50000
N_EDGES = 800000
D = 96
DPAD = 128          # fp16 gather element (256B, dma_gather alignment)
NCORES = 8
PER = N_NODES // NCORES          # 6250 dests per core
CHUNK = 64                       # dests per one-hot / matmul slice
NCHUNK = (PER + CHUNK - 1) // CHUNK   # 98 chunks (last short: 42)
WIN = 512                        # dests per PSUM window (8 chunks)
NWIN = (PER + WIN - 1) // WIN    # 13 windows
CPW = WIN // CHUNK               # chunks per window
SPLIT = 32768                    # int16 index limit for dma_gather
GT = 128                         # edges per tile
GE = 96                          # gathered elements per edge (192B of the 256B row)
CT = 8                           # tiles per dma_gather call (ring limit ~1024 idxs)
NQ = 4                           # SWDGE queues
PERPAD = 6272                    # 49 * 128, padded node count per core
NDCH = PERPAD // 128             # dense-tail chunks of 128 nodes
NEG_SLOPE = 0.01
DGRP = 4                         # chunks per dense-tail group

FP16 = mybir.dt.float16
F32 = mybir.dt.float32
I16 = mybir.dt.int16


def _win_chunks(w):
    return range(CPW * w, min(CPW * w + CPW, NCHUNK))


# ---------------------------------------------------------------- host prep
def _preprocess(a_rows, a_cols, a_vals):
    """Per-core edge layout with core-uniform tile counts.

    Edges are bucketed by (core, chunk-of-128-dests, int16 stream) and
    padded so every core shares the per-(chunk, stream) tile count
    T[k, s] (max over cores, >=1).  Tile order is (window, stream,
    chunk)-major so gather calls get long single-stream runs.
    """
    a_rows = np.asarray(a_rows).astype(np.int64)
    a_cols = np.asarray(a_cols).astype(np.int64)
    a_vals = np.asarray(a_vals).astype(np.float32)

    order = np.argsort(a_rows, kind="stable")
    r_s, c_s, v_s = a_rows[order], a_cols[order], a_vals[order]
    core_s = r_s // PER
    dloc_s = r_s % PER
    chunk_s = dloc_s // CHUNK
    col_s = dloc_s % CHUNK
    stream_s = (c_s >= SPLIT).astype(np.int64)

    counts = np.zeros((NCORES, NCHUNK, 2), dtype=np.int64)
    buckets = [[[None, None] for _ in range(NCHUNK)] for _ in range(NCORES)]
    for c in range(NCORES):
        m = core_s == c
        ch, st = chunk_s[m], stream_s[m]
        src, val, col = c_s[m], v_s[m], col_s[m]
        for k in range(NCHUNK):
            mk = ch == k
            for s in range(2):
                mm = mk & (st == s)
                idx = src[mm] - (SPLIT if s else 0)
                buckets[c][k][s] = (idx, val[mm], col[mm])
                counts[c, k, s] = mm.sum()

    T = np.zeros((NCHUNK, 2), dtype=np.int64)
    for k in range(NCHUNK):
        for s in range(2):
            T[k, s] = max(1, int(np.ceil(counts[:, k, s].max() / GT)))

    per_core = []
    for c in range(NCORES):
        idx_parts, val_parts, col_parts = [], [], []
        for w in range(NWIN):
            for s in range(2):
                for k in _win_chunks(w):
                    idx, val, col = buckets[c][k][s]
                    n_pad = int(T[k, s]) * GT
                    pad = n_pad - len(idx)
                    idx_parts.append(np.concatenate([idx, np.zeros(pad, np.int64)]))
                    val_parts.append(np.concatenate([val, np.zeros(pad, np.float32)]))
                    col_parts.append(np.concatenate([col, np.zeros(pad, np.int64)]))
        per_core.append(dict(
            idx=np.concatenate(idx_parts),
            val=np.concatenate(val_parts).astype(np.float32),
            col=np.concatenate(col_parts),
        ))
    return T, per_core


def _build_call_plan(T):
    """Tile order is (window, stream, chunk)-major.  Returns:
    calls: list of (stream, tile_start, n_tiles) single-stream runs <= CT
    tmap:  (chunk, stream, j) -> global tile index
    runs:  (window, stream) -> (tile_start, n_tiles) for one-hot batching
    n_tiles total.
    """
    calls = []
    tmap = {}
    runs = {}
    t = 0
    for w in range(NWIN):
        for s in range(2):
            r0 = t
            for k in _win_chunks(w):
                for j in range(int(T[k, s])):
                    tmap[(k, s, j)] = t
                    t += 1
            runs[(w, s)] = (r0, t - r0)
            done = r0
            while done < t:
                kk = min(CT, t - done)
                calls.append((s, done, kk, w))
                done += kk
    return calls, tmap, runs, t


def _wrap_idx16(idx_all, calls):
    """Per-call 16-partition-wrapped int16 index tiles, concatenated.
    Call k with n_tiles tiles occupies columns [8*tile_start, 8*(start+n))
    of a [128, 8*TT] int16 array (8 cols per tile: 128/16)."""
    TT = len(idx_all) // GT
    out = np.zeros((128, 8 * TT), dtype=np.int16)
    for s, t0, nt, w in calls:
        chunk = idx_all[t0 * GT:(t0 + nt) * GT].astype(np.int16)
        wrapped = chunk.reshape(-1, 16).T          # [16, nt*8]
        out[:, t0 * 8:(t0 + nt) * 8] = np.tile(wrapped, (8, 1))
    return out


# ---------------------------------------------------------------- builder
_CACHE = {}
_LAST_RESULT = None


def _build_program(T, calls, tmap, runs, TT):
    nc = bacc.Bacc("TRN2", target_bir_lowering=False, debug=False,
                   num_devices=NCORES, num_swdge_queues=NQ)

    ego_pad = nc.dram_tensor("ego_pad", [N_NODES, DPAD], FP16, kind="ExternalInput")
    idx16 = nc.dram_tensor("idx16", [128, 8 * TT], I16, kind="ExternalInput")
    vals = nc.dram_tensor("vals", [128, TT], FP16, kind="ExternalInput")
    cols = nc.dram_tensor("cols", [128, TT], FP16, kind="ExternalInput")
    iota = nc.dram_tensor("iota", [128, CHUNK], FP16, kind="ExternalInput")
    egot = nc.dram_tensor("egot", [D + 1, PERPAD], FP16, kind="ExternalInput")
    w1t = nc.dram_tensor("w1t", [D + 1, D], FP16, kind="ExternalInput")
    w2t = nc.dram_tensor("w2t", [D + 1, D], FP16, kind="ExternalInput")
    out = nc.dram_tensor("out", [PERPAD, D], F32, kind="ExternalOutput")

    tile2call = {}
    for ci, (s, t0, nt, w) in enumerate(calls):
        for j in range(nt):
            tile2call[t0 + j] = (ci, j)

    max_run = max(n for (_, n) in runs.values())

    with tile.TileContext(nc) as tc:
        with tc.tile_pool(name="const", bufs=1) as constp, \
             tc.tile_pool(name="gath", bufs=20) as gathp, \
             tc.tile_pool(name="oh", bufs=4) as ohp, \
             tc.tile_pool(name="pw", bufs=3, space="PSUM") as pwp, \
             tc.tile_pool(name="pd", bufs=2, space="PSUM") as pdp, \
             tc.tile_pool(name="act", bufs=2) as actp, \
             tc.tile_pool(name="big", bufs=1) as bigp:

            nc.gpsimd.load_library(library_config.mlp)

            # ---- idx in four staged loads on the sync HWDGE queue
            # (nothing else on it), so gather call 0 starts after ~10us
            # and the rest streams behind; all other constants go via
            # the scalar engine's HWDGE queue.
            bounds = []
            for b in [runs[(0, 1)][0], runs[(1, 0)][0], runs[(2, 0)][0]]:
                if b not in bounds:
                    bounds.append(b)
            bounds.append(TT)
            idx_tiles = []
            prev = 0
            for b in bounds:
                tl = constp.tile([128, 8 * (b - prev)], I16)
                nc.sync.dma_start(tl[:], idx16[:, 8 * prev:8 * b])
                idx_tiles.append((prev, b, tl))
                prev = b

            def idx_ap(t0, nt):
                for lo, hi, tl in idx_tiles:
                    if lo <= t0 < hi:
                        assert t0 + nt <= hi
                        return tl[:, (t0 - lo) * 8:(t0 - lo + nt) * 8]
                raise AssertionError(t0)

            iota_sb = constp.tile([128, CHUNK], FP16)
            nc.scalar.dma_start(iota_sb[:], iota[:])
            w1t_sb = constp.tile([D + 1, D], FP16)
            nc.scalar.dma_start(w1t_sb[:], w1t[:])
            w2t_sb = constp.tile([D + 1, D], FP16)
            nc.scalar.dma_start(w2t_sb[:], w2t[:])
            val_sb = constp.tile([128, TT], FP16)
            nc.scalar.dma_start(val_sb[:], vals[:])
            col_sb = constp.tile([128, TT], FP16)
            nc.scalar.dma_start(col_sb[:], cols[:])
            # egot streamed per window so window 0's epilogue isn't
            # gated on the full 1.2MB load
            egot_sb = bigp.tile([D + 1, PERPAD], FP16)
            for w in range(NWIN):
                c0 = w * WIN
                ce = min(PERPAD, c0 + WIN)
                nc.scalar.dma_start(egot_sb[:, c0:ce], egot[:, c0:ce])

            sumxt = bigp.tile([D + 1, PERPAD], FP16)
            bixt = bigp.tile([D + 1, PERPAD], FP16)

            # ones rows for the bias augmentation; zero the padded tail
            # columns up front so the dense tail can run per window
            nc.vector.memset(sumxt[D:D + 1, :], 1.0)
            nc.vector.memset(bixt[D:D + 1, :], 1.0)
            if PERPAD > PER:
                nc.vector.memset(sumxt[:D, PER:], 0.0)
                nc.vector.memset(bixt[:D, PER:], 0.0)

            # ---- gather calls (issued in order; Tile double-buffers)
            # 256B elems: smaller (192B) elems measured ~40% lower DMA
            # throughput per queue, a net loss.
            gath_tiles = [None] * len(calls)
            for ci, (s, t0, nt, w) in enumerate(calls):
                g = gathp.tile([128, CT, DPAD], FP16, tag="gath")
                src_ap = ego_pad[:SPLIT, :] if s == 0 else ego_pad[SPLIT:, :]
                nc.gpsimd.dma_gather(
                    g[:, :nt, :], src_ap, idx_ap(t0, nt),
                    nt * GT, nt * GT, DPAD, queue_num=ci % NQ,
                )
                gath_tiles[ci] = g

            # ---- per-window: batched one-hot build + per-chunk accumulation
            for w in range(NWIN):
                nd = min(WIN, PER - w * WIN)
                pw = pwp.tile([128, WIN], F32, tag="pw")
                # one-hot stacks, one per stream run
                oh_run = {}
                for s in range(2):
                    r0, rn = runs[(w, s)]
                    oh = ohp.tile([128, max_run, CHUNK], FP16, tag="oh")
                    nc.vector.tensor_tensor(
                        oh[:, :rn, :],
                        iota_sb[:, None, :].broadcast_to([128, rn, CHUNK]),
                        col_sb[:, r0:r0 + rn, None].broadcast_to([128, rn, CHUNK]),
                        mybir.AluOpType.is_equal,
                    )
                    nc.vector.tensor_tensor(
                        oh[:, :rn, :], oh[:, :rn, :],
                        val_sb[:, r0:r0 + rn, None].broadcast_to([128, rn, CHUNK]),
                        mybir.AluOpType.mult,
                    )
                    oh_run[s] = (oh, r0)
                for k in _win_chunks(w):
                    c0 = (k - CPW * w) * CHUNK
                    n_a, n_b = int(T[k, 0]), int(T[k, 1])
                    for jj in range(n_a + n_b):
                        s, j = (0, jj) if jj < n_a else (1, jj - n_a)
                        t = tmap[(k, s, j)]
                        ci, slot = tile2call[t]
                        g = gath_tiles[ci]
                        oh, r0 = oh_run[s]
                        nc.tensor.matmul(
                            pw[:, c0:c0 + CHUNK], g[:, slot, :],
                            oh[:, t - r0, :],
                            start=(jj == 0), stop=(jj == n_a + n_b - 1),
                        )
                # sideT window -> sumXt / biXt (feature-major)
                c0 = w * WIN
                nc.vector.tensor_tensor(
                    sumxt[:D, c0:c0 + nd], egot_sb[:D, c0:c0 + nd], pw[:D, :nd],
                    mybir.AluOpType.add,
                )
                nc.vector.tensor_tensor(
                    bixt[:D, c0:c0 + nd], egot_sb[:D, c0:c0 + nd], pw[:D, :nd],
                    mybir.AluOpType.mult,
                )

                # ---- dense tail for this window's nodes (DGRP=4 dense
                # chunks of 128 = exactly one 512-node window)
                g0 = w * DGRP
                ng = min(DGRP, NDCH - g0)
                p1 = pdp.tile([128, DGRP, D], F32, tag="pd1")
                p2 = pdp.tile([128, DGRP, D], F32, tag="pd2")
                for i in range(ng):
                    c0 = (g0 + i) * 128
                    nc.tensor.matmul(p1[:, i, :], sumxt[:, c0:c0 + 128],
                                     w1t_sb[:], start=True, stop=True)
                    nc.tensor.matmul(p2[:, i, :], bixt[:, c0:c0 + 128],
                                     w2t_sb[:], start=True, stop=True)
                s1 = actp.tile([128, DGRP, D], F32, tag="s1")
                nc.vector.tensor_scalar_mul(s1[:, :ng, :], p1[:, :ng, :], NEG_SLOPE)
                a1 = actp.tile([128, DGRP, D], F32, tag="a1")
                nc.vector.tensor_tensor(a1[:, :ng, :], s1[:, :ng, :], p1[:, :ng, :],
                                        mybir.AluOpType.max)
                s2 = actp.tile([128, DGRP, D], F32, tag="s2")
                nc.vector.tensor_scalar_mul(s2[:, :ng, :], p2[:, :ng, :], NEG_SLOPE)
                a2 = actp.tile([128, DGRP, D], F32, tag="a2")
                nc.vector.tensor_tensor(a2[:, :ng, :], s2[:, :ng, :], p2[:, :ng, :],
                                        mybir.AluOpType.max)
                out_g = actp.tile([128, DGRP, D], F32, tag="outg")
                nc.vector.tensor_tensor(out_g[:, :ng, :], a1[:, :ng, :],
                                        a2[:, :ng, :], mybir.AluOpType.add)
                nc.sync.dma_start(
                    out.rearrange("(k p) f -> p k f", p=128)[:, g0:g0 + ng, :],
                    out_g[:, :ng, :])

    nc.compile()
    return nc


# ---------------------------------------------------------------- entry
def kernel(ego, a_vals, W1, b1, W2, b2, a_rows, a_cols):
    ego = np.asarray(ego, dtype=np.float32)
    a_vals = np.asarray(a_vals, dtype=np.float32)
    W1 = np.asarray(W1, dtype=np.float32)
    b1 = np.asarray(b1, dtype=np.float32)
    W2 = np.asarray(W2, dtype=np.float32)
    b2 = np.asarray(b2, dtype=np.float32)
    a_rows_i = np.asarray(a_rows)
    a_cols_i = np.asarray(a_cols)

    T, per_core = _preprocess(a_rows_i, a_cols_i, a_vals)
    calls, tmap, runs, TT = _build_call_plan(T)

    key = (tuple(T.ravel().tolist()),)
    if key not in _CACHE:
        _CACHE[key] = _build_program(T, calls, tmap, runs, TT)
    nc = _CACHE[key]

    # shared inputs
    ego_pad = np.zeros((N_NODES, DPAD), dtype=np.float16)
    ego_pad[:, :D] = ego.astype(np.float16)
    iota_np = np.tile(np.arange(CHUNK, dtype=np.float32).astype(np.float16),
                      (128, 1))
    w1t_np = np.vstack([W1.T, b1[None, :]]).astype(np.float16)
    w2t_np = np.vstack([W2.T, b2[None, :]]).astype(np.float16)

    in_maps = []
    for c in range(NCORES):
        pc = per_core[c]
        idx16_np = _wrap_idx16(pc["idx"], calls)
        val_np = pc["val"].reshape(TT, GT).T.astype(np.float16)
        col_np = np.ascontiguousarray(
            pc["col"].astype(np.float16).reshape(TT, GT).T)
        egot_np = np.zeros((D + 1, PERPAD), dtype=np.float16)
        egot_np[:D, :PER] = ego[c * PER:(c + 1) * PER].T.astype(np.float16)
        egot_np[D, :] = 1.0
        in_maps.append({
            "ego_pad": ego_pad, "idx16": idx16_np,
            "vals": val_np, "cols": col_np, "iota": iota_np,
            "egot": egot_np, "w1t": w1t_np, "w2t": w2t_np,
        })

    res = bass_utils.run_bass_kernel_spmd(
        nc, in_maps, core_ids=list(range(NCORES)))
    global _LAST_RESULT
    _LAST_RESULT = res

    out = np.empty((N_NODES, D), dtype=np.float32)
    for c in range(NCORES):
        out[c * PER:(c + 1) * PER] = res.results[c]["out"][:PER]
    return out
